# revision 46
# baseline (speedup 1.0000x reference)
"""Trainium2 Bass kernel for CaiT talking-heads attention.

B=8 batch, N=1024 tokens, DIM=512, 8 heads x 64. Data-parallel: one batch
element per NeuronCore (8 cores).

Per-core algorithm:
  x^T via PE transpose (is_transpose mode, bf16)
  Q^T = w_q^T x^T, K^T = w_k^T x^T (feature-major), V = x w_v (token-major)
  for g in heads:                       # mixed-pre head index
    Qs_g = Q^T scaled rows by mix_pre[h(c),g]/8   (folds mix_pre + scale)
    S'^T_g = K^T.T-contracted vs Qs_g   # [j, i] tiles, K=512 contraction
    P_g = exp(S'^T_g)                   # softmax w/o max-sub (|S'| ~ < 6)
    V'_g = V * mix_post[g, head(col)]   (folds mix_post)
    out += (P_g @ V'_g) / rowsum(P_g)   # rowsum via ones-matmul piggyback
  y = out @ w_out + b_out  (out PE-transposed so it feeds lhsT directly)

Dtypes: x/w_q/w_k/w_v/w_out stream in as bf16 (halves HBM traffic, DMA is a
shared serial resource); scores run f32r x f32r; P/V' run bf16 x bf16; the
OUT accumulator stays f32r; y streams out bf16. Measured end-to-end rel err
~5e-3 vs the f32 reference.

Scheduling: all weight DMAs on the Act DGE queue in need-order (x on SP);
x^T transposes start per 128-row block as DMA lands; Qs scaling runs on the
Act engine, V' scaling on the Pool engine; the V projection slots between the
first head's two score blocks; OUT transposes and the output projection
interleave into the last head's PV chains; mpo/bias replicate on-chip via
partition_broadcast instead of DMAing 2.3 MB.
"""

import numpy as np
import ml_dtypes

import concourse.bass as bass
import concourse.bacc as bacc
import concourse.mybir as mybir
from concourse.bass_utils import run_bass_kernel_spmd
from concourse.masks import make_identity
from concourse.tile import TileContext

P = 128
N = 1024
DIM = 512
H = 8
DH = 64
F32 = mybir.dt.float32
F32R = mybir.dt.float32r
BF16 = mybir.dt.bfloat16

IB = N // P    # 8 token blocks
CC = DIM // P  # 4 feature chunks
NCORES = 8


def build_bass():
    nc = bacc.Bacc("TRN2")

    # x arrives host-shuffled to [P, IB, DIM] (partition-major) so paired
    # token blocks stream in single DMAs with matching AP iteration order
    x_d = nc.dram_tensor("x", [P, IB, DIM], BF16, kind="ExternalInput")
    wq_d = nc.dram_tensor("wq", [DIM, DIM], BF16, kind="ExternalInput")
    wk_d = nc.dram_tensor("wk", [DIM, DIM], BF16, kind="ExternalInput")
    wv_d = nc.dram_tensor("wv", [DIM, DIM], BF16, kind="ExternalInput")
    wout_d = nc.dram_tensor("wout", [DIM, DIM], BF16, kind="ExternalInput")
    # mp[p, cc*8+g] = mix_pre[(cc*128+p)//64, g] / 8
    mp_d = nc.dram_tensor("mp", [P, CC * H], F32, kind="ExternalInput")
    # mpo_s[0, h*512 + g*64+d] = mix_post[h, g]; replicated on-chip
    mpo_d = nc.dram_tensor("mpo", [1, H * DIM], F32R, kind="ExternalInput")
    # b_out is added on the host: y here is OUT @ w_out only
    y_d = nc.dram_tensor("y", [N, DIM], BF16, kind="ExternalOutput")

    with TileContext(nc) as tc:
        with (
            tc.tile_pool(name="persist", bufs=1) as pp,
            tc.tile_pool(name="ph01", bufs=1) as p01,
            tc.tile_pool(name="ph2", bufs=2) as p2,
            tc.tile_pool(name="ph34", bufs=1) as p34,
            tc.tile_pool(name="ps2", bufs=5, space="PSUM") as psp,
            tc.tile_pool(name="psr", bufs=1, space="PSUM") as psr,
        ):
            QT = pp.tile([P, CC, N], F32R)   # QT[p,cc,i] = q[i, cc*128+p]
            KT = pp.tile([P, CC, N], F32R)
            V = pp.tile([P, IB, DIM], F32R)  # V[p,jb,gd] = v[jb*128+p, gd]
            OUT = pp.tile([P, IB, DIM], F32R)
            mp = pp.tile([P, CC * H], F32)
            mpo_s = pp.tile([1, H * DIM], F32R)
            mpo = pp.tile([P, H, DIM], F32R)
            wout = pp.tile([P, CC, DIM], BF16)

            # ---- DMA issue: x and late tensors on the SP queue; wq/wk/mp on
            # the Act queue (short, so Act's SEQ frees up for Qs scaling
            # early).  All transfers share one serial HBM pipe, so the issue
            # order is the need order. ----
            # x blocks round-robin over four DGE queues: the 900ns DMA
            # completion semaphores then propagate in parallel
            # Pool SWDGE generation is ~1us per DMA, so it gets fewer blocks
            xsb = []
            chunks = [(0, 1, nc.sync), (1, 3, nc.gpsimd), (3, 5, nc.sync),
                      (5, 7, nc.gpsimd), (7, 8, nc.sync)]
            for lo, hi, eng in chunks:
                t = p01.tile([P, hi - lo, DIM], BF16, tag=f"xsb{lo}")
                eng.dma_start(t[:], x_d[:, lo:hi, :])
                for b in range(lo, hi):
                    xsb.append(t[:, b - lo, :])
            wq = p01.tile([P, CC, DIM], BF16)
            wk = p01.tile([P, CC, DIM], BF16)
            wv = p01.tile([P, CC, DIM], BF16)
            for c in range(CC):
                nc.scalar.dma_start(wq[:, c, :], wq_d[c * P:(c + 1) * P, :])
            for c in range(CC):
                nc.scalar.dma_start(wk[:, c, :], wk_d[c * P:(c + 1) * P, :])
            nc.scalar.dma_start(mp[:], mp_d[:])
            for c in range(CC):
                nc.sync.dma_start(wv[:, c, :], wv_d[c * P:(c + 1) * P, :])
            nc.sync.dma_start(mpo_s[:], mpo_d[:])
            for c in range(CC):
                nc.scalar.dma_start(wout[:, c, :], wout_d[c * P:(c + 1) * P, :])
            # replicate the tiny broadcast operand on the idle Pool engine
            nc.gpsimd.partition_broadcast(mpo[:], mpo_s[:])

            ident0 = pp.tile([P, P], F32)
            make_identity(nc, ident0)
            identr = pp.tile([P, P], F32R)
            nc.vector.tensor_copy(identr[:], ident0[:])
            ident = identr[:]
            identb = pp.tile([P, P], BF16)
            nc.vector.tensor_copy(identb[:], ident0[:])
            ones0 = pp.tile([P, 8], BF16)
            nc.vector.memset(ones0, 1.0)
            ones = ones0[:]


            xT = p01.tile([P, CC, N], BF16)  # xT[p,fc,i] = x[i, fc*128+p]

            def transpose_block(b):
                # 4 transposes into one bf16 psum tile (disjoint regions),
                # drained by a single wide copy
                pt = psp.tile([P, CC, P], BF16, tag="pst", bufs=2)
                for fc in range(CC):
                    nc.tensor.matmul(
                        pt[:, fc, :], xsb[b][:, fc * P:(fc + 1) * P], identb,
                        is_transpose=True,
                        start=(fc == 0), stop=(fc == CC - 1),
                        skip_group_check=True,
                    )
                nc.vector.tensor_copy(xT[:, :, b * P:(b + 1) * P], pt[:])

            def proj_T(dst, w, ih, interleave={}):
                isl = slice(ih * 512, (ih + 1) * 512)
                for cc in range(CC):
                    pq = psp.tile([P, DIM], F32, tag="ps")
                    for fc in range(CC):
                        nc.tensor.matmul(
                            pq, w[:, fc, cc * P:(cc + 1) * P], xT[:, fc, isl],
                            start=(fc == 0), stop=(fc == CC - 1),
                        )
                    nc.vector.tensor_copy(dst[:, cc, isl], pq)
                    for tb in interleave.get(cc, ()):
                        transpose_block(tb)

            for b in (0, 1, 2, 3):
                transpose_block(b)
            proj_T(QT, wq, 0, interleave={0: (6, 7), 1: (4, 5)})
            proj_T(QT, wq, 1)
            proj_T(KT, wk, 0)
            proj_T(KT, wk, 1)

            def emit_v():
                for jb in range(IB):
                    pv = psp.tile([P, DIM], F32, tag="ps")
                    for fc in range(CC):
                        nc.tensor.matmul(
                            pv, xT[:, fc, jb * P:(jb + 1) * P], wv[:, fc, :],
                            start=(fc == 0), stop=(fc == CC - 1),
                        )
                    nc.vector.tensor_copy(V[:, jb, :], pv)

            OT = p34.tile([P, CC, N], BF16)

            def out_transpose(bs):
                # OUT[:, b, :] -> OT[:, gc, b-block] once head g=7 done.
                # All four transposes land in one psum bank (disjoint column
                # ranges, accumulate-into-zeroed), drained by a single copy.
                for b in bs:
                    pt = psp.tile([P, CC, P], F32, tag="pst", bufs=2)
                    ptr = pt.bitcast(F32R)
                    for gc in range(CC):
                        nc.tensor.matmul(
                            ptr[:, gc, :],
                            OUT[:, b, gc * P:(gc + 1) * P], ident,
                            is_transpose=True,
                            start=(gc == 0), stop=(gc == CC - 1),
                            skip_group_check=True,
                        )
                    dst = OT[:, :, b * P:(b + 1) * P]
                    if b % 2 == 0:
                        nc.vector.tensor_copy(dst, ptr[:])
                    else:
                        nc.scalar.copy(dst, ptr[:])

            def emit_proj(bs, py34):
                for b in bs:
                    py = psp.tile([P, DIM], F32, tag="ps")
                    for gc in range(CC):
                        nc.tensor.matmul(
                            py, OT[:, gc, b * P:(b + 1) * P], wout[:, gc, :],
                            start=(gc == 0), stop=(gc == CC - 1),
                        )
                    ysb = py34.tile([P, DIM], BF16, tag="y")
                    # bias is added on the host; alternate copy engines +
                    # DGE queues so the last blocks drain in parallel
                    if b % 2 == 0:
                        nc.vector.tensor_copy(ysb[:], py)
                        nc.sync.dma_start(y_d[b * P:(b + 1) * P, :], ysb)
                    else:
                        nc.scalar.copy(ysb[:], py)
                        nc.scalar.dma_start(y_d[b * P:(b + 1) * P, :], ysb)

            # ---- per mixed-head scores+softmax+PV ----
            with tc.tile_pool(name="y34", bufs=2) as py34:
                for h in range(H):
                    # Qs on the Act engine; free dim 1024 spans both i-halves
                    Qs = p2.tile([P, CC, N], F32R, tag="qs")
                    for cc in range(CC):
                        nc.scalar.mul(
                            Qs[:, cc, :], QT[:, cc, :],
                            mp[:, cc * H + h:cc * H + h + 1],
                        )
                    # V' on the Pool engine (DVE for h=0: tighter deadline;
                    # emitted after the V copies below to keep DVE in order)
                    Vp = p2.tile([P, IB, DIM], BF16, tag="vp")
                    if h > 0:
                        for jb in range(IB):
                            nc.gpsimd.tensor_mul(
                                out=Vp[:, jb, :], in0=V[:, jb, :],
                                in1=mpo[:, h, :],
                            )
                    PTs = []
                    for ih in range(2):
                        isl = slice(ih * 512, (ih + 1) * 512)
                        PT = p2.tile([P, IB, 512], BF16, tag="pt")
                        PTs.append(PT)
                        for jb in range(IB):
                            ps = psp.tile([P, DIM], F32, tag="ps")
                            for cc in range(CC):
                                nc.tensor.matmul(
                                    ps, KT[:, cc, jb * P:(jb + 1) * P],
                                    Qs[:, cc, isl],
                                    start=(cc == 0), stop=(cc == CC - 1),
                                )
                            nc.scalar.activation(
                                PT[:, jb, :], ps,
                                mybir.ActivationFunctionType.Exp,
                            )
                        if h == 0 and ih == 0:
                            # V projection + h=0 V' slot in here, hidden
                            # under the first score block
                            emit_v()
                            for jb in range(IB):
                                nc.vector.tensor_mul(
                                    out=Vp[:, jb, :], in0=V[:, jb, :],
                                    in1=mpo[:, 0, :],
                                )
                    def pv_rowsum(ibs):
                        PT = PTs[ibs // 4]
                        il = ibs % 4
                        pr = psr.tile([P, 8], F32, tag="pr")
                        # rowsum chain first: its reciprocal clears the DVE
                        # queue while the PV chain still runs on the PE
                        for jb in range(IB):
                            nc.tensor.matmul(
                                pr, PT[:, jb, il * P:(il + 1) * P], ones,
                                start=(jb == 0), stop=(jb == IB - 1),
                            )
                        rr = p2.tile([P, 1], F32, tag="rr")
                        nc.vector.reciprocal(rr, pr[:, 0:1])
                        return rr

                    def pv_po(ibs, rr, csl=slice(0, DIM)):
                        PT = PTs[ibs // 4]
                        il = ibs % 4
                        ncols = csl.stop - csl.start
                        po = psp.tile([P, ncols], F32, tag="ps")
                        for jb in range(IB):
                            nc.tensor.matmul(
                                po, PT[:, jb, il * P:(il + 1) * P],
                                Vp[:, jb, csl],
                                start=(jb == 0), stop=(jb == IB - 1),
                            )
                        if h == 0:
                            nc.vector.tensor_scalar_mul(
                                OUT[:, ibs, csl], po, rr)
                        else:
                            nc.vector.scalar_tensor_tensor(
                                out=OUT[:, ibs, csl], in0=po, scalar=rr,
                                in1=OUT[:, ibs, csl],
                                op0=mybir.AluOpType.mult,
                                op1=mybir.AluOpType.add,
                            )

                    def pv_chain(ibs):
                        rr = pv_rowsum(ibs)
                        pv_po(ibs, rr)

                    if h < H - 1:
                        for ibs in range(IB):
                            pv_chain(ibs)
                    else:
                        # last head: thread OUT transposes (T) and output
                        # projections (P) between the PV chains (C) so only
                        # the last block's T/P trails the final chain
                        for step in ("C0 C1 C2 T0 C3 T1 P0 C4 T2 P1 C5 T3 "
                                     "P2 C6 T4 P3 T5 P4 P5 T6").split():
                            b = int(step[1])
                            if step[0] == "C":
                                pv_chain(b)
                            elif step[0] == "T":
                                out_transpose([b])
                            else:
                                emit_proj([b], py34)
                        # block 7 runs in column halves so its transposes,
                        # OT copies, projection, and DMA pipeline tightly
                        rr7 = pv_rowsum(7)
                        pv_po(7, rr7, slice(0, 256))
                        pv_po(7, rr7, slice(256, DIM))
                        pt7 = psp.tile([P, CC, P], F32, tag="pst", bufs=2)
                        pt7r = pt7.bitcast(F32R)

                        def t7(gcs, last):
                            for gc in gcs:
                                nc.tensor.matmul(
                                    pt7r[:, gc, :],
                                    OUT[:, 7, gc * P:(gc + 1) * P], ident,
                                    is_transpose=True,
                                    start=(gc == 0),
                                    stop=(last and gc == gcs[-1]),
                                    skip_group_check=True,
                                )

                        t7([0, 1], False)
                        nc.vector.tensor_copy(
                            OT[:, 0:2, 7 * P:8 * P], pt7r[:, 0:2, :])
                        emit_proj([6], py34)
                        t7([2, 3], True)
                        nc.scalar.copy(
                            OT[:, 2:4, 7 * P:8 * P], pt7r[:, 2:4, :])
                        # final projection: half-width ysb copies and DMAs
                        # drain on both engines/queues in parallel
                        py = psp.tile([P, DIM], F32, tag="ps")
                        for gc in range(CC):
                            nc.tensor.matmul(
                                py, OT[:, gc, 7 * P:8 * P], wout[:, gc, :],
                                start=(gc == 0), stop=(gc == CC - 1),
                            )
                        ysb = py34.tile([P, DIM], BF16, tag="y")
                        nc.vector.tensor_copy(ysb[:, 0:256], py[:, 0:256])
                        nc.scalar.copy(ysb[:, 256:DIM], py[:, 256:DIM])
                        nc.sync.dma_start(y_d[7 * P:N, 0:256], ysb[:, 0:256])
                        nc.scalar.dma_start(
                            y_d[7 * P:N, 256:DIM], ysb[:, 256:DIM])

    nc.finalize()
    return nc


_NC_CACHE = None
TRACE = False
LAST_RESULT = None


def kernel(x, w_q, w_kv, mix_pre, mix_post, w_out, b_out):
    global _NC_CACHE
    x = np.asarray(x, np.float32)
    w_q = np.asarray(w_q, np.float32)
    w_kv = np.asarray(w_kv, np.float32)
    mix_pre = np.asarray(mix_pre, np.float32)
    mix_post = np.asarray(mix_post, np.float32)
    w_out = np.asarray(w_out, np.float32)
    b_out = np.asarray(b_out, np.float32)

    bf = ml_dtypes.bfloat16
    w_k = np.ascontiguousarray(w_kv[:, :DIM].astype(bf))
    w_v = np.ascontiguousarray(w_kv[:, DIM:].astype(bf))
    w_q8 = np.ascontiguousarray(w_q.astype(bf))
    w_o8 = np.ascontiguousarray(w_out.astype(bf))

    # mp[p, cc*8+g] = mix_pre[head of channel cc*128+p, g] * (1/sqrt(64))
    ch = (np.arange(DIM) // DH)  # head of channel
    mp = np.zeros((P, CC * H), np.float32)
    for cc in range(CC):
        for g in range(H):
            mp[:, cc * H + g] = mix_pre[ch[cc * P:(cc + 1) * P], g] * 0.125
    # mpo_s[0, h*512+col] = mix_post[h, col//64]
    mpo_s = np.ascontiguousarray(
        np.repeat(mix_post, DH, axis=1).reshape(1, H * DIM).astype(np.float32)
    )

    if _NC_CACHE is None:
        _NC_CACHE = build_bass()
    nc = _NC_CACHE

    base = {
        "wq": w_q8, "wk": w_k, "wv": w_v, "wout": w_o8,
        "mp": mp, "mpo": mpo_s,
    }
    in_maps = [
        dict(base, x=np.ascontiguousarray(
            x[b].astype(bf).reshape(IB, P, DIM).transpose(1, 0, 2)))
        for b in range(NCORES)
    ]
    global LAST_RESULT
    res = run_bass_kernel_spmd(
        nc, in_maps, core_ids=list(range(NCORES)), trace=TRACE,
        trace_cores=list(range(NCORES)) if TRACE else None,
    )
    LAST_RESULT = res
    out = np.stack(
        [np.asarray(res.results[b]["y"], dtype=np.float32)
         for b in range(NCORES)], axis=0)
    return out + b_out[None, None, :]
